# revision 4
# baseline (speedup 1.0000x reference)
"""Trainium2 Bass kernel: MultiHeadLatentAttention prefill (B=2, T=2048, D=2048,
H=16, HD=128, KVH=4, QL=1536, KVL=512).

Sharding: 8 cores = (batch b in {0,1}) x (kv-head group g in {0..3}).
Each core handles one batch element and the 4 q-heads of one kv head.
Host folds rms weights into up-projections, folds the (position = kv-head
index) K-rope rotation into Wkr, sums the 8 partial outputs at the end.

Device dataflow (feature-major activations [feat, T], bf16 matmuls, fp32 PSUM):
  ql_T = WdqT.T @ x_T            (rms-normalized in place via ones-matmul stats)
  c_T  = WdkvT.T @ x_T           (rms-normalized in place)
  K_T  = WkrT.T @ x_T            (rope pre-folded on host)
  Q_T  = WuqT.T @ ql_T ; Qr_T = rope(WqrT.T @ ql_T)  (rope = C*x + S*pairswap(x))
  Kabs_T = WukT.T @ c_T          (the absorbed-Wuk trick, shared by 4 heads)
  V    = (c_T slices).T @ WuvT   (token-major)
  per head: S_T[s,t] = Kabs_T[:,s].T @ Q_T + K_T[:,s].T @ Qr_T   (keys on partitions)
            E = exp(S_T/16) * causal_mask ;  Z = ones.T @ E ;  ctx_T = V.T @ E
            ctx_T *= 1/Z
  out_T = WoutT.T @ ctx_T        (partial over heads; host sums over g)
"""

import numpy as np
import ml_dtypes

B, T, D = 2, 2048, 2048
H, HD, KVH = 16, 128, 4
QL, KVL = 1536, 512
G = KVH                  # core groups per batch
HPG = H // KVH           # q heads per group
NCORES = B * G
TS = 512                 # free-dim tile
NT = T // TS             # 4
DCH = D // 128           # 16
QLCH = QL // 128         # 12
CCH = KVL // 128         # 4
SCH = T // 128           # 16
EPS = 1e-6
SM_SCALE = 1.0 / 16.0    # 1/sqrt(2*HD)
BF16 = ml_dtypes.bfloat16

_CACHE = {}
LAST_RESULTS = None


def _build_program(reps=1):
    import concourse.bacc as bacc
    import concourse.tile as tile
    from concourse import mybir
    from concourse.bass import ts

    bf = mybir.dt.bfloat16
    f32 = mybir.dt.float32
    AF = mybir.ActivationFunctionType
    SWAP_MASK = [i ^ 1 for i in range(32)]

    nc = bacc.Bacc("TRN2", target_bir_lowering=False, debug=False)

    xT = nc.dram_tensor("x_T", [D, T], bf, kind="ExternalInput")
    wdqT = nc.dram_tensor("wdqT", [D, QL], bf, kind="ExternalInput")
    wdkvT = nc.dram_tensor("wdkvT", [D, KVL], bf, kind="ExternalInput")
    wkrT = nc.dram_tensor("wkrT", [D, HD], bf, kind="ExternalInput")
    wuqT = nc.dram_tensor("wuqT", [QL, HPG * HD], bf, kind="ExternalInput")
    wqrT = nc.dram_tensor("wqrT", [QL, HPG * HD], bf, kind="ExternalInput")
    wukT = nc.dram_tensor("wukT", [KVL, HD], bf, kind="ExternalInput")
    wuvT = nc.dram_tensor("wuvT", [KVL, HD], bf, kind="ExternalInput")
    woutT = nc.dram_tensor("woutT", [HPG * HD, D], bf, kind="ExternalInput")
    ctab = nc.dram_tensor("ctab", [HD, T], bf, kind="ExternalInput")
    stab = nc.dram_tensor("stab", [HD, T], bf, kind="ExternalInput")
    outT = nc.dram_tensor("out_T", [D, T], f32, kind="ExternalOutput")

    with tile.TileContext(nc) as tc:
      for _rep in range(reps):
        with (
            tc.tile_pool(name="A", bufs=1) as A,
            tc.tile_pool(name="QLP", bufs=1) as QLP,
        ):
            c_sb = A.tile([128, CCH, T], bf)
            k_sb = A.tile([128, T], bf)
            kabs_sb = A.tile([128, T], bf)
            v_sb = A.tile([128, SCH, HD], bf)
            wuk_sb = A.tile([128, CCH, HD], bf)
            wuv_sb = A.tile([128, CCH, HD], bf)
            ctab_sb = A.tile([128, T], bf)
            stab_sb = A.tile([128, T], bf)
            ones_sb = A.tile([128, 1], bf)
            masks_sb = A.tile([128, 4, TS], bf)
            sq_row = A.tile([1, T], f32)
            sc_row = A.tile([1, T], f32)
            eps_sb = A.tile([1, 1], f32)
            ql_sb = QLP.tile([128, QLCH, T], bf)

            nc.vector.memset(ones_sb, 1.0)
            nc.vector.memset(eps_sb, EPS)
            # causal masks for diagonal blocks: mask_r[x, y] = 1 if y >= x + 128*r
            for r in range(4):
                nc.vector.memset(masks_sb[:, r, :], 1.0)
                nc.gpsimd.affine_select(
                    out=masks_sb[:, r, :],
                    in_=masks_sb[:, r, :],
                    pattern=[[1, TS]],
                    compare_op=mybir.AluOpType.is_ge,
                    fill=0.0,
                    base=-(128 * r),
                    channel_multiplier=-1,
                )
            nc.sync.dma_start(
                out=wuk_sb, in_=wukT.ap().rearrange("(c p) d -> p c d", p=128)
            )
            nc.sync.dma_start(
                out=wuv_sb, in_=wuvT.ap().rearrange("(c p) d -> p c d", p=128)
            )
            nc.sync.dma_start(out=ctab_sb, in_=ctab.ap())
            nc.sync.dma_start(out=stab_sb, in_=stab.ap())

            # ---------------- phase 1: latent projections from x ----------------
            with (
                tc.tile_pool(name="PH1", bufs=1) as P1,
                tc.tile_pool(name="P1S", bufs=3) as P1S,
                tc.tile_pool(name="PP1", bufs=3, space="PSUM") as PP1,
                tc.tile_pool(name="PZ1", bufs=2, space="PSUM") as PZ1,
            ):
                x_sb = P1.tile([128, DCH, T], bf)
                sqq_bc = P1.tile([128, T], f32)
                scc_bc = P1.tile([128, T], f32)
                xT_r = xT.ap().rearrange("(c p) t -> c p t", p=128)
                for d in range(DCH):
                    nc.sync.dma_start(out=x_sb[:, d, :], in_=xT_r[d])

                for m in range(QLCH + CCH + 1):
                    if m < QLCH:
                        wsrc = wdqT.ap()[:, m * 128:(m + 1) * 128]
                    elif m < QLCH + CCH:
                        cc = m - QLCH
                        wsrc = wdkvT.ap()[:, cc * 128:(cc + 1) * 128]
                    else:
                        wsrc = wkrT.ap()
                    w_t = P1S.tile([128, DCH, 128], bf, tag="wstream")
                    nc.sync.dma_start(
                        out=w_t, in_=wsrc.rearrange("(c p) f -> p c f", p=128)
                    )
                    for t in range(NT):
                        ps = PP1.tile([128, TS], f32, tag="pp")
                        for d in range(DCH):
                            nc.tensor.matmul(
                                ps,
                                lhsT=w_t[:, d, :],
                                rhs=x_sb[:, d, ts(t, TS)],
                                start=(d == 0),
                                stop=(d == DCH - 1),
                            )
                        if m < QLCH:
                            dst = ql_sb[:, m, ts(t, TS)]
                        elif m < QLCH + CCH:
                            dst = c_sb[:, m - QLCH, ts(t, TS)]
                        else:
                            dst = k_sb[:, ts(t, TS)]
                        nc.vector.tensor_copy(dst, ps)

                # rms statistics via squares + ones-matmul (sum over partitions)
                for nch, src, row, nrm in (
                    (QLCH, ql_sb, sq_row, QL),
                    (CCH, c_sb, sc_row, KVL),
                ):
                    for t in range(NT):
                        psz = PZ1.tile([1, TS], f32, tag="pz")
                        for m in range(nch):
                            sq = P1S.tile([128, TS], bf, tag="wstream")
                            nc.vector.tensor_mul(
                                sq, src[:, m, ts(t, TS)], src[:, m, ts(t, TS)]
                            )
                            nc.tensor.matmul(
                                psz,
                                lhsT=ones_sb,
                                rhs=sq,
                                start=(m == 0),
                                stop=(m == nch - 1),
                            )
                        # row = sqrt(ss/nrm + eps)
                        nc.scalar.activation(
                            row[0:1, ts(t, TS)],
                            psz,
                            AF.Sqrt,
                            bias=eps_sb[0:1, 0:1],
                            scale=1.0 / nrm,
                        )
                    nc.vector.reciprocal(row[0:1, :], row[0:1, :])
                nc.gpsimd.partition_broadcast(sqq_bc, sq_row[0:1, :])
                nc.gpsimd.partition_broadcast(scc_bc, sc_row[0:1, :])
                for m in range(QLCH):
                    nc.vector.tensor_mul(ql_sb[:, m, :], ql_sb[:, m, :], sqq_bc)
                for cc in range(CCH):
                    nc.vector.tensor_mul(c_sb[:, cc, :], c_sb[:, cc, :], scc_bc)

                # Kabs (absorbed Wuk) and token-major V from normalized c
                for t in range(NT):
                    ps = PP1.tile([128, TS], f32, tag="pp")
                    for cc in range(CCH):
                        nc.tensor.matmul(
                            ps,
                            lhsT=wuk_sb[:, cc, :],
                            rhs=c_sb[:, cc, ts(t, TS)],
                            start=(cc == 0),
                            stop=(cc == CCH - 1),
                        )
                    nc.vector.tensor_copy(kabs_sb[:, ts(t, TS)], ps)
                for s in range(SCH):
                    ps = PP1.tile([128, HD], f32, tag="ppv")
                    for cc in range(CCH):
                        nc.tensor.matmul(
                            ps,
                            lhsT=c_sb[:, cc, s * 128:(s + 1) * 128],
                            rhs=wuv_sb[:, cc, :],
                            start=(cc == 0),
                            stop=(cc == CCH - 1),
                        )
                    nc.vector.tensor_copy(v_sb[:, s, :], ps)

            # ---------------- phases 2+3: per-head Q/Qr projection + attention ----
            with tc.tile_pool(name="P3B", bufs=1) as P3B:
                ctx_sb = P3B.tile([128, HPG, T], bf)
                with (
                    tc.tile_pool(name="P3S", bufs=2) as P3S,
                    tc.tile_pool(name="EP", bufs=8) as EP,
                    tc.tile_pool(name="TMPP", bufs=4) as TMPP,
                    tc.tile_pool(name="PQK", bufs=2, space="PSUM") as PQK,
                    tc.tile_pool(name="PSC", bufs=2, space="PSUM") as PSC,
                    tc.tile_pool(name="PCT", bufs=2, space="PSUM") as PCT,
                    tc.tile_pool(name="PZ3", bufs=2, space="PSUM") as PZ3,
                ):
                    for h in range(HPG):
                        wuq_t = P3S.tile([128, QLCH, HD], bf, tag="wuq")
                        wqr_t = P3S.tile([128, QLCH, HD], bf, tag="wqr")
                        nc.sync.dma_start(
                            out=wuq_t,
                            in_=wuqT.ap()[:, h * HD:(h + 1) * HD].rearrange(
                                "(c p) f -> p c f", p=128
                            ),
                        )
                        nc.sync.dma_start(
                            out=wqr_t,
                            in_=wqrT.ap()[:, h * HD:(h + 1) * HD].rearrange(
                                "(c p) f -> p c f", p=128
                            ),
                        )
                        qh_sb = P3S.tile([128, T], bf, tag="qh")
                        qrh_sb = P3S.tile([128, T], bf, tag="qrh")
                        for t in range(NT):
                            psq = PQK.tile([128, TS], f32, tag="pqk")
                            for m in range(QLCH):
                                nc.tensor.matmul(
                                    psq,
                                    lhsT=wuq_t[:, m, :],
                                    rhs=ql_sb[:, m, ts(t, TS)],
                                    start=(m == 0),
                                    stop=(m == QLCH - 1),
                                )
                            nc.vector.tensor_copy(qh_sb[:, ts(t, TS)], psq)
                            psr = PQK.tile([128, TS], f32, tag="pqk")
                            for m in range(QLCH):
                                nc.tensor.matmul(
                                    psr,
                                    lhsT=wqr_t[:, m, :],
                                    rhs=ql_sb[:, m, ts(t, TS)],
                                    start=(m == 0),
                                    stop=(m == QLCH - 1),
                                )
                            # rope: qrh = psr*C + pairswap(psr)*S
                            tsw = TMPP.tile([128, TS], f32, tag="tmp")
                            nc.vector.stream_shuffle(tsw, psr, SWAP_MASK)
                            t1 = TMPP.tile([128, TS], f32, tag="tmp")
                            nc.vector.tensor_mul(t1, psr, ctab_sb[:, ts(t, TS)])
                            t2 = TMPP.tile([128, TS], f32, tag="tmp")
                            nc.vector.tensor_mul(t2, tsw, stab_sb[:, ts(t, TS)])
                            nc.vector.tensor_add(qrh_sb[:, ts(t, TS)], t1, t2)

                        for j in range(NT):
                            n_s = 4 * (j + 1)
                            psz = PZ3.tile([1, TS], f32, tag="pz3")
                            pctx = PCT.tile([128, TS], f32, tag="pct")
                            for s in range(n_s):
                                pss = PSC.tile([128, TS], f32, tag="psc")
                                nc.tensor.matmul(
                                    pss,
                                    lhsT=kabs_sb[:, s * 128:(s + 1) * 128],
                                    rhs=qh_sb[:, ts(j, TS)],
                                    start=True,
                                    stop=False,
                                )
                                nc.tensor.matmul(
                                    pss,
                                    lhsT=k_sb[:, s * 128:(s + 1) * 128],
                                    rhs=qrh_sb[:, ts(j, TS)],
                                    start=False,
                                    stop=True,
                                )
                                e_t = EP.tile([128, TS], bf, tag="e")
                                nc.scalar.activation(e_t, pss, AF.Exp, scale=SM_SCALE)
                                if s >= 4 * j:
                                    nc.vector.tensor_mul(
                                        e_t, e_t, masks_sb[:, s - 4 * j, :]
                                    )
                                nc.tensor.matmul(
                                    psz,
                                    lhsT=ones_sb,
                                    rhs=e_t,
                                    start=(s == 0),
                                    stop=(s == n_s - 1),
                                )
                                nc.tensor.matmul(
                                    pctx,
                                    lhsT=v_sb[:, s, :],
                                    rhs=e_t,
                                    start=(s == 0),
                                    stop=(s == n_s - 1),
                                )
                            zrow = TMPP.tile([1, TS], f32, tag="zrow")
                            nc.vector.tensor_copy(zrow, psz)
                            zinv = TMPP.tile([1, TS], f32, tag="zrow")
                            nc.vector.reciprocal(zinv, zrow)
                            zbc = TMPP.tile([128, TS], f32, tag="zbc")
                            nc.gpsimd.partition_broadcast(zbc, zinv[0:1, :])
                            nc.vector.tensor_mul(
                                ctx_sb[:, h, ts(j, TS)], pctx, zbc
                            )

                # ---------------- phase 4: output projection ----------------
                with (
                    tc.tile_pool(name="P4", bufs=3) as P4,
                    tc.tile_pool(name="PP4", bufs=3, space="PSUM") as PP4,
                ):
                    for e in range(DCH):
                        wo_t = P4.tile([128, HPG, 128], bf, tag="wo")
                        nc.sync.dma_start(
                            out=wo_t,
                            in_=woutT.ap()[:, e * 128:(e + 1) * 128].rearrange(
                                "(c p) f -> p c f", p=128
                            ),
                        )
                        for t in range(NT):
                            ps = PP4.tile([128, TS], f32, tag="pp4")
                            for q in range(HPG):
                                nc.tensor.matmul(
                                    ps,
                                    lhsT=wo_t[:, q, :],
                                    rhs=ctx_sb[:, q, ts(t, TS)],
                                    start=(q == 0),
                                    stop=(q == HPG - 1),
                                )
                            o_t = P4.tile([128, TS], f32, tag="ot")
                            nc.vector.tensor_copy(o_t, ps)
                            nc.sync.dma_start(
                                out=outT.ap()[e * 128:(e + 1) * 128, ts(t, TS)],
                                in_=o_t,
                            )

    nc.compile()
    return nc


def _get_program():
    if "nc" not in _CACHE:
        _CACHE["nc"] = _build_program()
    return _CACHE["nc"]


def _host_prep(inputs):
    """Fold weights on the host and build the 8 per-core input maps."""
    x = np.asarray(inputs["x"], np.float32)
    Wdq = np.asarray(inputs["Wdq"], np.float32)
    qw = np.asarray(inputs["q_norm_w"], np.float32)
    Wuq = np.asarray(inputs["Wuq"], np.float32) * qw[None, :]
    Wqr = np.asarray(inputs["Wqr"], np.float32) * qw[None, :]
    Wdkv = np.asarray(inputs["Wdkv"], np.float32)
    kvw = np.asarray(inputs["kv_norm_w"], np.float32)
    Wuk = np.asarray(inputs["Wuk"], np.float32) * kvw[None, :]
    Wuv = np.asarray(inputs["Wuv"], np.float32) * kvw[None, :]
    Wkr = np.asarray(inputs["Wkr"], np.float32)
    Wout = np.asarray(inputs["Wout"], np.float32)

    inv = 1.0 / (10000.0 ** (np.arange(0, HD, 2, dtype=np.float32) / HD))
    f = np.arange(T, dtype=np.float32)[None, :] * inv[:, None]   # [64, T]
    cosT, sinT = np.cos(f), np.sin(f)
    Ctab = np.repeat(cosT, 2, axis=0)                            # [128, T]
    Stab = np.repeat(sinT, 2, axis=0)
    Stab[0::2, :] *= -1.0                                        # pair-swap sign

    fH = np.arange(KVH, dtype=np.float32)[None, :] * inv[:, None]  # [64, KVH]
    cosH, sinH = np.cos(fH), np.sin(fH)

    def bft(a):
        return np.ascontiguousarray(a).astype(BF16)

    wdqT = bft(Wdq.T)
    wdkvT = bft(Wdkv.T)
    wukT = bft(Wuk.T)
    ctab_b = bft(Ctab)
    stab_b = bft(Stab)

    in_maps = []
    for b in range(B):
        x_T = bft(x[b].T)
        for g in range(G):
            # fold K-rope (fixed rotation per kv-head index) into Wkr
            Wkr_g = Wkr[g * HD:(g + 1) * HD, :]
            we, wo = Wkr_g[0::2, :], Wkr_g[1::2, :]
            c_g, s_g = cosH[:, g][:, None], sinH[:, g][:, None]
            Wkr_eff = np.empty_like(Wkr_g)
            Wkr_eff[0::2, :] = we * c_g - wo * s_g
            Wkr_eff[1::2, :] = we * s_g + wo * c_g

            in_maps.append(
                dict(
                    x_T=x_T,
                    wdqT=wdqT,
                    wdkvT=wdkvT,
                    wkrT=bft(Wkr_eff.T),
                    wuqT=bft(Wuq[g * HPG * HD:(g + 1) * HPG * HD].T),
                    wqrT=bft(Wqr[g * HPG * HD:(g + 1) * HPG * HD].T),
                    wukT=wukT,
                    wuvT=bft(Wuv[g * HD:(g + 1) * HD].T),
                    woutT=bft(Wout[:, g * HPG * HD:(g + 1) * HPG * HD].T),
                    ctab=ctab_b,
                    stab=stab_b,
                )
            )
    return in_maps


def kernel(**inputs):
    global LAST_RESULTS
    from concourse import bass_utils

    nc = _get_program()
    in_maps = _host_prep(inputs)
    res = bass_utils.run_bass_kernel_spmd(
        nc, in_maps, core_ids=list(range(NCORES))
    )
    LAST_RESULTS = res
    out = np.zeros((B, T, D), np.float32)
    for i, r in enumerate(res.results):
        out[i // G] += r["out_T"].T
    return out


# revision 22
# speedup vs baseline: 56.7453x; 56.7453x over previous
"""Trainium2 Bass kernel: MultiHeadLatentAttention prefill (B=2, T=2048, D=2048,
H=16, HD=128, KVH=4, QL=1536, KVL=512).

Sharding: 8 cores = (batch b in {0,1}) x (kv-head group g in {0..3}).
Each core handles one batch element and the 4 q-heads of one kv head.
Host folds rms weights into up-projections, folds the (position = kv-head
index) K-rope rotation into Wkr, sums the 8 partial outputs at the end.

Device dataflow (feature-major activations [feat, T], bf16 matmuls, fp32 PSUM):
  ql_T = WdqT.T @ x_T ; c_T = WdkvT.T @ x_T ; K_T = WkrT.T @ x_T (rope folded)
  rms scales for ql/c are computed via ones-matmul column sums of squares and
  applied at consumer epilogues (never in place - no pipeline barrier):
    Q_T  = (WuqT.T @ ql_T) * sq[t]
    Qr_T = rope(WqrT.T @ ql_T) with rope tables pre-scaled by sq[t]
           (rope(x) = x*C + pairswap(x)*S, pairswap via stream_shuffle)
    Kabs_T = (WukT.T @ c_T) * sc[t]   (absorbed-Wuk trick, shared by 4 heads)
    V    = (c_T slices).T @ WuvT * sc[s]  (token-major; per-partition scale)
  per head: S_T[s,t] = Kabs_T[:,s].T @ Q_T + K_T[:,s].T @ Qr_T (keys on parts)
            E = exp(S_T/16) * causal_mask ;  Z = ones.T @ E ;  ctx_T = V.T @ E
            ctx_T *= 1/Z
  out_T = WoutT.T @ ctx_T  (partial over heads; host sums over g)
"""

import numpy as np
import ml_dtypes

B, T, D = 2, 2048, 2048
H, HD, KVH = 16, 128, 4
QL, KVL = 1536, 512
G = KVH                  # core groups per batch
HPG = H // KVH           # q heads per group
NCORES = B * G
TS = 512                 # free-dim tile
NT = T // TS             # 4
DCH = D // 128           # 16
QLCH = QL // 128         # 12
CCH = KVL // 128         # 4
SCH = T // 128           # 16
EPS = 1e-6
SM_SCALE = 1.0 / 16.0    # 1/sqrt(2*HD)
BF16 = ml_dtypes.bfloat16

_CACHE = {}
LAST_RESULTS = None


def _build_program(reps=1):
    import concourse.bacc as bacc
    import concourse.tile as tile
    from concourse import mybir
    from concourse.bass import ts

    bf = mybir.dt.bfloat16
    f32 = mybir.dt.float32
    AF = mybir.ActivationFunctionType
    SWAP_MASK = [i ^ 1 for i in range(32)]

    nc = bacc.Bacc("TRN2", target_bir_lowering=False, debug=False)

    xT = nc.dram_tensor("x_T", [D, T], bf, kind="ExternalInput")
    wdqT = nc.dram_tensor("wdqT", [D, QL], bf, kind="ExternalInput")
    wdkvT = nc.dram_tensor("wdkvT", [D, KVL], bf, kind="ExternalInput")
    wkrT = nc.dram_tensor("wkrT", [D, HD], bf, kind="ExternalInput")
    wuqT = nc.dram_tensor("wuqT", [QL, HPG * HD], bf, kind="ExternalInput")
    wqrT = nc.dram_tensor("wqrT", [QL, HPG * HD], bf, kind="ExternalInput")
    wukT = nc.dram_tensor("wukT", [KVL, HD], bf, kind="ExternalInput")
    wuvT = nc.dram_tensor("wuvT", [KVL, HD], bf, kind="ExternalInput")
    woutT = nc.dram_tensor("woutT", [HPG * HD, D], bf, kind="ExternalInput")
    ctab = nc.dram_tensor("ctab", [HD, T], bf, kind="ExternalInput")
    stab = nc.dram_tensor("stab", [HD, T], bf, kind="ExternalInput")
    outT = nc.dram_tensor("out_T", [D, T], f32, kind="ExternalOutput")

    # phase-1 projection schedule: c chunks first (so the rms-scale chain for
    # the kv side completes early), then K, then ql chunks
    M_TOTAL = CCH + 1 + QLCH

    def proj_src(m):
        if m < CCH:
            return wdkvT.ap()[:, m * 128:(m + 1) * 128]
        if m == CCH:
            return wkrT.ap()
        return wdqT.ap()[:, (m - CCH - 1) * 128:(m - CCH) * 128]

    with tile.TileContext(nc) as tc:
      for _rep in range(reps):
        with (
            tc.tile_pool(name="A", bufs=1) as A,
            tc.tile_pool(name="QLP", bufs=1) as QLP,
        ):
            c_sb = A.tile([128, CCH, T], bf)
            k_sb = A.tile([128, T], bf)
            kabs_sb = A.tile([128, T], bf)
            v_sb = A.tile([128, SCH, HD], bf)
            wuk_sb = A.tile([128, CCH, HD], bf)
            wuv_sb = A.tile([128, CCH, HD], bf)
            ones_sb = A.tile([128, 1], bf)
            sq_row = A.tile([1, T], f32)
            sc_row = A.tile([1, T], f32)
            sc_col = A.tile([128, SCH], f32)   # column form of sc (for V)
            eps_sb = A.tile([1, 1], f32)
            ql_sb = QLP.tile([128, QLCH, T], bf)
            sqq_bc = QLP.tile([128, T], f32)   # broadcast of 1/rms(ql)

            # ---------------- phase 1: latent projections from x -------------
            with (
                tc.tile_pool(name="PH1", bufs=1) as P1,
                tc.tile_pool(name="P1S", bufs=3) as P1S,
                tc.tile_pool(name="DRS", bufs=1, space="DRAM") as DRS,
                tc.tile_pool(name="PP1", bufs=3, space="PSUM") as PP1,
                tc.tile_pool(name="PZ1", bufs=4, space="PSUM") as PZ1,
            ):
                scc_bc = P1.tile([128, T], f32)    # broadcast of 1/rms(c)
                xT_r = xT.ap().rearrange("(c p) t -> c p t", p=128)
                x_sb = []
                for d in range(DCH):
                    xd = P1.tile([128, T], bf, tag=f"x{d}", name=f"x{d}")
                    nc.gpsimd.dma_start(out=xd, in_=xT_r[d])
                    x_sb.append(xd)

                w_ts = []
                for m in range(M_TOTAL):
                    w_t = P1S.tile([128, DCH, 128], bf, tag="wstream")
                    nc.sync.dma_start(
                        out=w_t, in_=proj_src(m).rearrange("(c p) f -> p c f", p=128)
                    )
                    w_ts.append(w_t)
                    if m == 1:
                        # small constants after the first two weight slices
                        nc.vector.memset(ones_sb, 1.0)
                        nc.vector.memset(eps_sb, EPS)
                        nc.sync.dma_start(
                            out=wuk_sb,
                            in_=wukT.ap().rearrange("(c p) d -> p c d", p=128),
                        )
                        nc.sync.dma_start(
                            out=wuv_sb,
                            in_=wuvT.ap().rearrange("(c p) d -> p c d", p=128),
                        )

                # psz[t] accumulate sum-of-squares across chunks (c then ql)
                psz_c = [None] * NT
                psz_q = [None] * NT
                for m in range(M_TOTAL):
                    w_t = w_ts[m]
                    is_c = m < CCH
                    is_k = m == CCH
                    for t in range(NT):
                        ps = PP1.tile([128, TS], f32, tag="pp")
                        for d in range(DCH):
                            nc.tensor.matmul(
                                ps,
                                lhsT=w_t[:, d, :],
                                rhs=x_sb[d][:, ts(t, TS)],
                                start=(d == 0),
                                stop=(d == DCH - 1),
                            )
                        if is_c:
                            dst = c_sb[:, m, ts(t, TS)]
                        elif is_k:
                            dst = k_sb[:, ts(t, TS)]
                        else:
                            dst = ql_sb[:, m - CCH - 1, ts(t, TS)]
                        nc.vector.tensor_copy(dst, ps)
                        if is_k:
                            continue
                        # interleaved rms stats on the bf16 copy
                        sq = P1S.tile([128, TS], bf, tag="wstream")
                        nc.vector.tensor_mul(sq, dst, dst)
                        psz = psz_c if is_c else psz_q
                        mi = m if is_c else m - CCH - 1
                        nch = CCH if is_c else QLCH
                        if mi == 0:
                            psz[t] = PZ1.tile([1, TS], f32, tag="pz", name="psz")
                        nc.tensor.matmul(
                            psz[t],
                            lhsT=ones_sb,
                            rhs=sq,
                            start=(mi == 0),
                            stop=(mi == nch - 1),
                        )
                        if mi == nch - 1:
                            row = sc_row if is_c else sq_row
                            nrm = KVL if is_c else QL
                            nc.scalar.activation(
                                row[0:1, ts(t, TS)],
                                psz[t],
                                AF.Sqrt,
                                bias=eps_sb[0:1, 0:1],
                                scale=1.0 / nrm,
                            )
                    # scale chains as soon as each row completes
                    if m == CCH - 1:
                        nc.vector.reciprocal(sc_row[0:1, :], sc_row[0:1, :])
                        nc.gpsimd.partition_broadcast(scc_bc, sc_row[0:1, :])
                        # column form of sc via DRAM round-trip
                        dr = DRS.tile([1, T], f32)
                        nc.sync.dma_start(out=dr, in_=sc_row[0:1, :])
                        nc.sync.dma_start(
                            out=sc_col,
                            in_=dr[:, :].rearrange("o (s p) -> (o p) s", p=128),
                        )
                    if m == M_TOTAL - 1:
                        nc.vector.reciprocal(sq_row[0:1, :], sq_row[0:1, :])
                        nc.gpsimd.partition_broadcast(sqq_bc, sq_row[0:1, :])

                # Kabs (absorbed Wuk) and token-major V from raw c + epilogue
                for t in range(NT):
                    ps = PP1.tile([128, TS], f32, tag="pp")
                    for cc in range(CCH):
                        nc.tensor.matmul(
                            ps,
                            lhsT=wuk_sb[:, cc, :],
                            rhs=c_sb[:, cc, ts(t, TS)],
                            start=(cc == 0),
                            stop=(cc == CCH - 1),
                        )
                    nc.vector.tensor_mul(
                        kabs_sb[:, ts(t, TS)], ps, scc_bc[:, ts(t, TS)]
                    )
                for s in range(SCH):
                    ps = PP1.tile([128, TS], f32, tag="pp")
                    for cc in range(CCH):
                        nc.tensor.matmul(
                            ps[:, 0:HD],
                            lhsT=c_sb[:, cc, s * 128:(s + 1) * 128],
                            rhs=wuv_sb[:, cc, :],
                            start=(cc == 0),
                            stop=(cc == CCH - 1),
                        )
                    nc.vector.tensor_scalar_mul(
                        v_sb[:, s, :], ps[:, 0:HD], sc_col[:, s:s + 1]
                    )

            # ---------------- phases 2+3: per-head Q/Qr + attention ----------
            with tc.tile_pool(name="P3B", bufs=1) as P3B:
                # per-(head, j) ctx tiles so phase 4 can start on early tiles
                ctx_sb = [
                    [
                        P3B.tile([128, TS], bf, tag=f"ctx{h}_{j}", name=f"ctx{h}_{j}")
                        for j in range(NT)
                    ]
                    for h in range(HPG)
                ]
                wout_sb = P3B.tile([128, HPG, T], bf)
                ctab_sb = P3B.tile([128, T], bf)
                stab_sb = P3B.tile([128, T], bf)
                masks_sb = P3B.tile([128, 4, TS], bf)
                for r in range(4):
                    nc.vector.memset(masks_sb[:, r, :], 1.0)
                    nc.gpsimd.affine_select(
                        out=masks_sb[:, r, :],
                        in_=masks_sb[:, r, :],
                        pattern=[[1, TS]],
                        compare_op=mybir.AluOpType.is_ge,
                        fill=0.0,
                        base=-(128 * r),
                        channel_multiplier=-1,
                    )
                with (
                    tc.tile_pool(name="P3S", bufs=2) as P3S,
                    tc.tile_pool(name="EP", bufs=8) as EP,
                    tc.tile_pool(name="TMPP", bufs=4) as TMPP,
                    tc.tile_pool(name="ZR", bufs=3) as ZR,
                    tc.tile_pool(name="PQK", bufs=2, space="PSUM") as PQK,
                    tc.tile_pool(name="PSC", bufs=2, space="PSUM") as PSC,
                    tc.tile_pool(name="PCT", bufs=2, space="PSUM") as PCT,
                    tc.tile_pool(name="PZ3", bufs=2, space="PSUM") as PZ3,
                ):
                    for h in range(HPG):
                        wuq_t = P3S.tile([128, QLCH, HD], bf, tag="wuq")
                        wqr_t = P3S.tile([128, QLCH, HD], bf, tag="wqr")
                        nc.sync.dma_start(
                            out=wuq_t,
                            in_=wuqT.ap()[:, h * HD:(h + 1) * HD].rearrange(
                                "(c p) f -> p c f", p=128
                            ),
                        )
                        nc.sync.dma_start(
                            out=wqr_t,
                            in_=wqrT.ap()[:, h * HD:(h + 1) * HD].rearrange(
                                "(c p) f -> p c f", p=128
                            ),
                        )
                        if h == 0:
                            # prefetch phase-3/4 constants behind head-0 weights
                            nc.sync.dma_start(out=ctab_sb, in_=ctab.ap())
                            nc.sync.dma_start(out=stab_sb, in_=stab.ap())
                            nc.gpsimd.dma_start(
                                out=wout_sb,
                                in_=woutT.ap().rearrange("(c p) e -> p c e", p=128),
                            )
                        qh_sb = P3S.tile([128, T], bf, tag="qh")
                        qrh_sb = P3S.tile([128, T], bf, tag="qrh")
                        for t in range(NT):
                            psq = PQK.tile([128, TS], f32, tag="pqk")
                            for m in range(QLCH):
                                nc.tensor.matmul(
                                    psq,
                                    lhsT=wuq_t[:, m, :],
                                    rhs=ql_sb[:, m, ts(t, TS)],
                                    start=(m == 0),
                                    stop=(m == QLCH - 1),
                                )
                            nc.vector.tensor_mul(
                                qh_sb[:, ts(t, TS)], psq, sqq_bc[:, ts(t, TS)]
                            )
                            psr = PQK.tile([128, TS], f32, tag="pqk")
                            for m in range(QLCH):
                                nc.tensor.matmul(
                                    psr,
                                    lhsT=wqr_t[:, m, :],
                                    rhs=ql_sb[:, m, ts(t, TS)],
                                    start=(m == 0),
                                    stop=(m == QLCH - 1),
                                )
                            # rope: qrh = (psr*C + pairswap(psr)*S) * sq
                            tsw = TMPP.tile([128, TS], f32, tag="tmp")
                            nc.vector.stream_shuffle(tsw, psr, SWAP_MASK)
                            t1 = TMPP.tile([128, TS], f32, tag="tmp")
                            nc.vector.tensor_mul(t1, psr, ctab_sb[:, ts(t, TS)])
                            t2 = TMPP.tile([128, TS], f32, tag="tmp")
                            nc.vector.tensor_mul(t2, tsw, stab_sb[:, ts(t, TS)])
                            t3 = TMPP.tile([128, TS], f32, tag="tmp")
                            nc.vector.tensor_add(t3, t1, t2)
                            nc.vector.tensor_mul(
                                qrh_sb[:, ts(t, TS)], t3, sqq_bc[:, ts(t, TS)]
                            )

                        for j in range(NT):
                            n_s = 4 * (j + 1)
                            psz = PZ3.tile([1, TS], f32, tag="pz3")
                            pctx = PCT.tile([128, TS], f32, tag="pct")
                            for s in range(n_s):
                                pss = PSC.tile([128, TS], f32, tag="psc")
                                nc.tensor.matmul(
                                    pss,
                                    lhsT=kabs_sb[:, s * 128:(s + 1) * 128],
                                    rhs=qh_sb[:, ts(j, TS)],
                                    start=True,
                                    stop=False,
                                )
                                nc.tensor.matmul(
                                    pss,
                                    lhsT=k_sb[:, s * 128:(s + 1) * 128],
                                    rhs=qrh_sb[:, ts(j, TS)],
                                    start=False,
                                    stop=True,
                                )
                                e_t = EP.tile([128, TS], bf, tag="e")
                                nc.scalar.activation(e_t, pss, AF.Exp, scale=SM_SCALE)
                                if s >= 4 * j:
                                    nc.vector.tensor_mul(
                                        e_t, e_t, masks_sb[:, s - 4 * j, :]
                                    )
                                nc.tensor.matmul(
                                    psz,
                                    lhsT=ones_sb,
                                    rhs=e_t,
                                    start=(s == 0),
                                    stop=(s == n_s - 1),
                                )
                                nc.tensor.matmul(
                                    pctx,
                                    lhsT=v_sb[:, s, :],
                                    rhs=e_t,
                                    start=(s == 0),
                                    stop=(s == n_s - 1),
                                )
                            zrow = ZR.tile([1, TS], f32, tag="zrow")
                            nc.vector.tensor_copy(zrow, psz)
                            zinv = ZR.tile([1, TS], f32, tag="zrow")
                            nc.vector.reciprocal(zinv, zrow)
                            zbc = TMPP.tile([128, TS], f32, tag="zbc")
                            nc.gpsimd.partition_broadcast(zbc, zinv[0:1, :])
                            nc.vector.tensor_mul(ctx_sb[h][j], pctx, zbc)

                # ---------------- phase 4: output projection -----------------
                with (
                    tc.tile_pool(name="P4", bufs=3) as P4,
                    tc.tile_pool(name="PP4", bufs=3, space="PSUM") as PP4,
                ):
                    for e in range(DCH):
                        o_t = P4.tile([128, T], f32, tag="ot")
                        for t in range(NT):
                            ps = PP4.tile([128, TS], f32, tag="pp4")
                            for q in range(HPG):
                                nc.tensor.matmul(
                                    ps,
                                    lhsT=wout_sb[:, q, e * 128:(e + 1) * 128],
                                    rhs=ctx_sb[q][t],
                                    start=(q == 0),
                                    stop=(q == HPG - 1),
                                )
                            nc.vector.tensor_copy(o_t[:, ts(t, TS)], ps)
                        nc.gpsimd.dma_start(
                            out=outT.ap()[e * 128:(e + 1) * 128, :], in_=o_t
                        )

    nc.compile()
    return nc


def _get_program():
    if "nc" not in _CACHE:
        _CACHE["nc"] = _build_program()
    return _CACHE["nc"]


def _host_prep(inputs):
    """Fold weights on the host and build the 8 per-core input maps."""
    x = np.asarray(inputs["x"], np.float32)
    Wdq = np.asarray(inputs["Wdq"], np.float32)
    qw = np.asarray(inputs["q_norm_w"], np.float32)
    Wuq = np.asarray(inputs["Wuq"], np.float32) * qw[None, :]
    Wqr = np.asarray(inputs["Wqr"], np.float32) * qw[None, :]
    Wdkv = np.asarray(inputs["Wdkv"], np.float32)
    kvw = np.asarray(inputs["kv_norm_w"], np.float32)
    Wuk = np.asarray(inputs["Wuk"], np.float32) * kvw[None, :]
    Wuv = np.asarray(inputs["Wuv"], np.float32) * kvw[None, :]
    Wkr = np.asarray(inputs["Wkr"], np.float32)
    Wout = np.asarray(inputs["Wout"], np.float32)

    inv = 1.0 / (10000.0 ** (np.arange(0, HD, 2, dtype=np.float32) / HD))
    f = np.arange(T, dtype=np.float32)[None, :] * inv[:, None]   # [64, T]
    cosT, sinT = np.cos(f), np.sin(f)
    Ctab = np.repeat(cosT, 2, axis=0)                            # [128, T]
    Stab = np.repeat(sinT, 2, axis=0)
    Stab[0::2, :] *= -1.0                                        # pair-swap sign

    fH = np.arange(KVH, dtype=np.float32)[None, :] * inv[:, None]  # [64, KVH]
    cosH, sinH = np.cos(fH), np.sin(fH)

    def bft(a):
        return np.ascontiguousarray(a).astype(BF16)

    wdqT = bft(Wdq.T)
    wdkvT = bft(Wdkv.T)
    wukT = bft(Wuk.T)
    ctab_b = bft(Ctab)
    stab_b = bft(Stab)

    in_maps = []
    for b in range(B):
        x_T = bft(x[b].T)
        for g in range(G):
            # fold K-rope (fixed rotation per kv-head index) into Wkr
            Wkr_g = Wkr[g * HD:(g + 1) * HD, :]
            we, wo = Wkr_g[0::2, :], Wkr_g[1::2, :]
            c_g, s_g = cosH[:, g][:, None], sinH[:, g][:, None]
            Wkr_eff = np.empty_like(Wkr_g)
            Wkr_eff[0::2, :] = we * c_g - wo * s_g
            Wkr_eff[1::2, :] = we * s_g + wo * c_g

            in_maps.append(
                dict(
                    x_T=x_T,
                    wdqT=wdqT,
                    wdkvT=wdkvT,
                    wkrT=bft(Wkr_eff.T),
                    wuqT=bft(Wuq[g * HPG * HD:(g + 1) * HPG * HD].T),
                    wqrT=bft(Wqr[g * HPG * HD:(g + 1) * HPG * HD].T),
                    wukT=wukT,
                    wuvT=bft(Wuv[g * HD:(g + 1) * HD].T),
                    woutT=bft(Wout[:, g * HPG * HD:(g + 1) * HPG * HD].T),
                    ctab=ctab_b,
                    stab=stab_b,
                )
            )
    return in_maps


def kernel(**inputs):
    global LAST_RESULTS
    from concourse import bass_utils

    nc = _get_program()
    in_maps = _host_prep(inputs)
    res = bass_utils.run_bass_kernel_spmd(
        nc, in_maps, core_ids=list(range(NCORES))
    )
    LAST_RESULTS = res
    out = np.zeros((B, T, D), np.float32)
    for i, r in enumerate(res.results):
        out[i // G] += r["out_T"].T
    return out
